# revision 6
# baseline (speedup 1.0000x reference)
"""Trainium2 Bass kernel for nn_CrossTransFormer_86526411145604.

Computation (b=4, C=1024, H=8 heads, dh=128, p=2048):
  Q = LeakyReLU(BN1(Wq @ Xq)), K = LeakyReLU(BN2(Wk @ Xk)), V = LeakyReLU(BN3(Wv @ Xq))
  per (b,h): S = Kh^T Vh / sqrt(dh); A = softmax_j(S); out[c,i] = sum_j A[i,j] Qh[c,j]

Sharding: 8 cores = (4 batches) x (2 head-groups of 4 heads). Each core gets
Xq[b], Xk[b] and the 512-channel slice of the (BN-scale-folded, transposed)
weights for its head group. All attention intermediates stay in SBUF; the
2048x2048 per-head softmax matrix is never materialized in HBM.

Layout trick: S is computed transposed (S^T[j,i] = Vh^T Kh) so the second
attention matmul needs no transposes: out[c,i] = sum_j Qt[j,c] P^T[j,i] with
Qt produced directly in [position, channel] layout by the Q branch
(lhsT = Xq chunk). Softmax row sums come from a ones-vector matmul; the
1/l normalization is broadcast across partitions with a rank-1 PE matmul.
"""

import math
import os

import numpy as np

C = 1024
H = 8
EPS = 1e-5
SLOPE = 0.1
B = 4
P = 2048
HG = 4            # heads per core
CB = 256          # branch column-block width
NCB = P // CB
IB = 512          # attention i-block width
NIB = P // IB
NKC = C // 128    # contraction chunks for the branch matmuls
NJC = P // 128    # j chunks for the attention contraction

_PROGRAM = None
LAST_RESULTS = None


def _build_program():
    import concourse.mybir as mybir
    import concourse.tile as tile
    from concourse import bacc

    f32 = mybir.dt.float32
    f32r = mybir.dt.float32r
    bf16 = mybir.dt.bfloat16
    LRELU = mybir.ActivationFunctionType.Prelu
    EXP = mybir.ActivationFunctionType.Exp

    nc = bacc.Bacc("TRN2", target_bir_lowering=False, debug=False)

    xq = nc.dram_tensor("xq", [C, P], f32r, kind="ExternalInput")
    xk = nc.dram_tensor("xk", [C, P], f32r, kind="ExternalInput")
    wq = nc.dram_tensor("wq", [C, 512], f32r, kind="ExternalInput")
    wk = nc.dram_tensor("wk", [C, 512], f32r, kind="ExternalInput")
    wv = nc.dram_tensor("wv", [C, 512], f32r, kind="ExternalInput")
    bq = nc.dram_tensor("bq", [1, 512], f32r, kind="ExternalInput")
    ones = nc.dram_tensor("ones", [1, 128], f32r, kind="ExternalInput")
    bk = nc.dram_tensor("bk", [128, HG], f32, kind="ExternalInput")
    bv = nc.dram_tensor("bv", [128, HG], f32, kind="ExternalInput")
    out = nc.dram_tensor("out", [512, P], f32, kind="ExternalOutput")

    sc = 1.0 / math.sqrt(C / H)

    with tile.TileContext(nc) as tc:
        with tc.tile_pool(name="wpool", bufs=1) as wpool, \
             tc.tile_pool(name="cpool", bufs=1) as cpool, \
             tc.tile_pool(name="apool", bufs=1) as apool, \
             tc.tile_pool(name="xpool", bufs=2) as xpool, \
             tc.tile_pool(name="ptpool", bufs=17) as ptpool, \
             tc.tile_pool(name="opool", bufs=2) as opool, \
             tc.tile_pool(name="pmm", bufs=4, space="PSUM") as pmm, \
             tc.tile_pool(name="pout", bufs=2, space="PSUM") as pout, \
             tc.tile_pool(name="psm", bufs=2, space="PSUM") as psm:

            wq_sb = wpool.tile([128, NKC, 512], f32r)
            nc.sync.dma_start(wq_sb[:], wq.ap().rearrange("(kc p) n -> p kc n", p=128))
            wk_sb = wpool.tile([128, NKC, 512], f32r)
            nc.sync.dma_start(wk_sb[:], wk.ap().rearrange("(kc p) n -> p kc n", p=128))
            wv_sb = wpool.tile([128, NKC, 512], f32r)
            nc.sync.dma_start(wv_sb[:], wv.ap().rearrange("(kc p) n -> p kc n", p=128))

            bq_sb = cpool.tile([1, 512], f32r)
            nc.sync.dma_start(bq_sb[:], bq.ap())
            bk_sb = cpool.tile([128, HG], f32)
            nc.sync.dma_start(bk_sb[:], bk.ap())
            bv_sb = cpool.tile([128, HG], f32)
            nc.sync.dma_start(bv_sb[:], bv.ap())

            ones_row = cpool.tile([1, 128], f32r)
            nc.sync.dma_start(ones_row[:], ones.ap())
            ones_col = cpool.tile([128, 1], bf16)
            nc.vector.memset(ones_col[:], 1.0)

            kh_sb = apool.tile([128, HG, P], f32r)
            vh_sb = apool.tile([128, HG, P], f32r)
            qt_sb = apool.tile([128, NJC, 512], bf16)

            xqv = xq.ap().rearrange("(kc p) i -> p kc i", p=128)
            xkv = xk.ap().rearrange("(kc p) i -> p kc i", p=128)

            # ---- branch phase: K, V (natural layout) and Q (transposed) ----
            for cb in range(NCB):
                cs = slice(cb * CB, (cb + 1) * CB)
                xq_t = xpool.tile([128, NKC, CB], f32r, tag="xq")
                nc.sync.dma_start(xq_t[:], xqv[:, :, cs])
                xk_t = xpool.tile([128, NKC, CB], f32r, tag="xk")
                nc.sync.dma_start(xk_t[:], xkv[:, :, cs])
                for hl in range(HG):
                    hs = slice(hl * 128, (hl + 1) * 128)
                    ps_k = pmm.tile([128, CB], f32, tag="mm")
                    for kc in range(NKC):
                        nc.tensor.matmul(ps_k[:], wk_sb[:, kc, hs], xk_t[:, kc, :],
                                         start=(kc == 0), stop=(kc == NKC - 1))
                    nc.scalar.activation(kh_sb[:, hl, cs], ps_k[:], LRELU,
                                         bias=bk_sb[:, hl:hl + 1], alpha=SLOPE)
                    ps_v = pmm.tile([128, CB], f32, tag="mm")
                    for kc in range(NKC):
                        nc.tensor.matmul(ps_v[:], wv_sb[:, kc, hs], xq_t[:, kc, :],
                                         start=(kc == 0), stop=(kc == NKC - 1))
                    nc.scalar.activation(vh_sb[:, hl, cs], ps_v[:], LRELU,
                                         bias=bv_sb[:, hl:hl + 1], alpha=SLOPE)
                for js in range(CB // 128):
                    ps_q = pmm.tile([128, 512], f32, tag="mm")
                    for kc in range(NKC):
                        nc.tensor.matmul(ps_q[:], xq_t[:, kc, js * 128:(js + 1) * 128],
                                         wq_sb[:, kc, :],
                                         start=(kc == 0), stop=False)
                    nc.tensor.matmul(ps_q[:], ones_row[:, 0:128], bq_sb[:],
                                     start=False, stop=True)
                    nc.scalar.activation(qt_sb[:, cb * (CB // 128) + js, :], ps_q[:],
                                         LRELU, alpha=SLOPE)

            # ---- attention phase, per local head and i-block ----
            for hl in range(HG):
                for ib in range(NIB):
                    ibs = slice(ib * IB, (ib + 1) * IB)
                    pts = []
                    for jm in range(NJC):
                        ps_s = pmm.tile([128, IB], f32, tag="mm")
                        nc.tensor.matmul(ps_s[:], vh_sb[:, hl, jm * 128:(jm + 1) * 128],
                                         kh_sb[:, hl, ibs], start=True, stop=True)
                        pt = ptpool.tile([128, IB], bf16, tag="pt")
                        nc.scalar.activation(pt[:], ps_s[:], EXP, scale=sc)
                        pts.append(pt)
                    ps_o = pout.tile([128, IB], f32)
                    for jc in range(NJC):
                        nc.tensor.matmul(ps_o[:], qt_sb[:, jc, hl * 128:(hl + 1) * 128],
                                         pts[jc][:], start=(jc == 0), stop=(jc == NJC - 1))
                    ps_l = psm.tile([1, IB], f32, tag="sm")
                    for jc in range(NJC):
                        nc.tensor.matmul(ps_l[:], ones_col[:], pts[jc][:],
                                         start=(jc == 0), stop=(jc == NJC - 1))
                    rc = opool.tile([1, IB], f32r, tag="rc")
                    with nc.allow_low_precision(reason="f32r holds full fp32 bits"):
                        nc.vector.reciprocal(rc[:], ps_l[:])
                    ps_b = psm.tile([128, IB], f32, tag="sm")
                    nc.tensor.matmul(ps_b[:], ones_row[:, 0:128], rc[:],
                                     start=True, stop=True)
                    rb = opool.tile([128, IB], f32, tag="rb")
                    nc.vector.tensor_copy(rb[:], ps_b[:])
                    ot = opool.tile([128, IB], f32, tag="ot")
                    nc.vector.tensor_mul(ot[:], ps_o[:], rb[:])
                    nc.sync.dma_start(out.ap()[hl * 128:(hl + 1) * 128, ibs], ot[:])

    nc.compile()
    return nc


def _get_program():
    global _PROGRAM
    if _PROGRAM is None:
        _PROGRAM = _build_program()
    return _PROGRAM


def kernel(Xq, Xk, Wq, Wk, Wv,
           gamma1, beta1, mean1, var1,
           gamma2, beta2, mean2, var2,
           gamma3, beta3, mean3, var3):
    global LAST_RESULTS
    from concourse.bass_utils import run_bass_kernel_spmd

    Xq = np.asarray(Xq, np.float32)
    Xk = np.asarray(Xk, np.float32)

    def fold(Wm, gamma, beta, mean, var):
        scale = np.asarray(gamma, np.float32) / np.sqrt(np.asarray(var, np.float32) + EPS)
        bias = np.asarray(beta, np.float32) - np.asarray(mean, np.float32) * scale
        Ws = np.asarray(Wm, np.float32) * scale[:, None]
        return Ws, bias

    Wq_s, b1 = fold(Wq, gamma1, beta1, mean1, var1)
    Wk_s, b2 = fold(Wk, gamma2, beta2, mean2, var2)
    Wv_s, b3 = fold(Wv, gamma3, beta3, mean3, var3)

    in_maps = []
    for core in range(8):
        b, hg = divmod(core, 2)
        sl = slice(hg * 512, (hg + 1) * 512)
        in_maps.append({
            "xq": np.ascontiguousarray(Xq[b]),
            "xk": np.ascontiguousarray(Xk[b]),
            "wq": np.ascontiguousarray(Wq_s[sl, :].T),
            "wk": np.ascontiguousarray(Wk_s[sl, :].T),
            "wv": np.ascontiguousarray(Wv_s[sl, :].T),
            "bq": np.ascontiguousarray(b1[sl].reshape(1, 512)),
            "ones": np.ones((1, 128), np.float32),
            "bk": np.ascontiguousarray(b2[sl].reshape(HG, 128).T),
            "bv": np.ascontiguousarray(b3[sl].reshape(HG, 128).T),
        })

    nc = _get_program()
    trace = os.environ.get("KERNEL_TRACE", "0") == "1"
    res = run_bass_kernel_spmd(nc, in_maps, core_ids=list(range(8)), trace=trace)
    LAST_RESULTS = res

    full = np.empty((B, C, P), np.float32)
    for core in range(8):
        b, hg = divmod(core, 2)
        full[b, hg * 512:(hg + 1) * 512, :] = res.results[core]["out"]
    return full


# revision 7
# speedup vs baseline: 1.0876x; 1.0876x over previous
"""Trainium2 Bass kernel for nn_CrossTransFormer_86526411145604.

Computation (b=4, C=1024, H=8 heads, dh=128, p=2048):
  Q = LeakyReLU(BN1(Wq @ Xq)), K = LeakyReLU(BN2(Wk @ Xk)), V = LeakyReLU(BN3(Wv @ Xq))
  per (b,h): S = Kh^T Vh / sqrt(dh); A = softmax_j(S); out[c,i] = sum_j A[i,j] Qh[c,j]

Sharding: 8 cores = (4 batches) x (2 head-groups of 4 heads). Each core gets
Xq[b], Xk[b] and the 512-channel slice of the (BN-scale-folded, transposed)
weights for its head group. All attention intermediates stay in SBUF; the
2048x2048 per-head softmax matrix is never materialized in HBM.

Layout trick: S is computed transposed (S^T[j,i] = Vh^T Kh) so the second
attention matmul needs no transposes: out[c,i] = sum_j Qt[j,c] P^T[j,i] with
Qt produced directly in [position, channel] layout by the Q branch
(lhsT = Xq chunk). Softmax row sums come from a ones-vector matmul; the
1/l normalization is broadcast across partitions with a rank-1 PE matmul.
"""

import math
import os

import numpy as np

C = 1024
H = 8
EPS = 1e-5
SLOPE = 0.1
B = 4
P = 2048
HG = 4            # heads per core
CB = 256          # branch column-block width
NCB = P // CB
IB = 512          # attention i-block width
NIB = P // IB
NKC = C // 128    # contraction chunks for the branch matmuls
NJC = P // 128    # j chunks for the attention contraction

_PROGRAM = None
LAST_RESULTS = None


def _build_program():
    import concourse.mybir as mybir
    import concourse.tile as tile
    from concourse import bacc

    f32 = mybir.dt.float32
    f32r = mybir.dt.float32r
    bf16 = mybir.dt.bfloat16
    LRELU = mybir.ActivationFunctionType.Prelu
    EXP = mybir.ActivationFunctionType.Exp

    nc = bacc.Bacc("TRN2", target_bir_lowering=False, debug=False)

    xq = nc.dram_tensor("xq", [C, P], f32r, kind="ExternalInput")
    xk = nc.dram_tensor("xk", [C, P], f32r, kind="ExternalInput")
    wq = nc.dram_tensor("wq", [C, 512], f32r, kind="ExternalInput")
    wk = nc.dram_tensor("wk", [C, 512], f32r, kind="ExternalInput")
    wv = nc.dram_tensor("wv", [C, 512], f32r, kind="ExternalInput")
    bq = nc.dram_tensor("bq", [1, 512], f32r, kind="ExternalInput")
    ones = nc.dram_tensor("ones", [1, 128], f32r, kind="ExternalInput")
    bk = nc.dram_tensor("bk", [128, HG], f32, kind="ExternalInput")
    bv = nc.dram_tensor("bv", [128, HG], f32, kind="ExternalInput")
    out = nc.dram_tensor("out", [512, P], f32, kind="ExternalOutput")

    sc = 1.0 / math.sqrt(C / H)

    with tile.TileContext(nc) as tc:
        with tc.tile_pool(name="wpool", bufs=1) as wpool, \
             tc.tile_pool(name="cpool", bufs=1) as cpool, \
             tc.tile_pool(name="apool", bufs=1) as apool, \
             tc.tile_pool(name="xpool", bufs=2) as xpool, \
             tc.tile_pool(name="ptpool", bufs=17) as ptpool, \
             tc.tile_pool(name="opool", bufs=2) as opool, \
             tc.tile_pool(name="pmm", bufs=2, space="PSUM") as pmm, \
             tc.tile_pool(name="pout", bufs=2, space="PSUM") as pout, \
             tc.tile_pool(name="psm", bufs=2, space="PSUM") as psm:

            wq_sb = wpool.tile([128, NKC, 512], f32r)
            nc.sync.dma_start(wq_sb[:], wq.ap().rearrange("(kc p) n -> p kc n", p=128))
            wk_sb = wpool.tile([128, NKC, 512], f32r)
            nc.sync.dma_start(wk_sb[:], wk.ap().rearrange("(kc p) n -> p kc n", p=128))
            wv_sb = wpool.tile([128, NKC, 512], f32r)
            nc.sync.dma_start(wv_sb[:], wv.ap().rearrange("(kc p) n -> p kc n", p=128))

            bq_sb = cpool.tile([1, 512], f32r)
            nc.sync.dma_start(bq_sb[:], bq.ap())
            bk_sb = cpool.tile([128, HG], f32)
            nc.sync.dma_start(bk_sb[:], bk.ap())
            bv_sb = cpool.tile([128, HG], f32)
            nc.sync.dma_start(bv_sb[:], bv.ap())

            ones_row = cpool.tile([1, 128], f32r)
            nc.sync.dma_start(ones_row[:], ones.ap())
            ones_col = cpool.tile([128, 1], bf16)
            nc.vector.memset(ones_col[:], 1.0)

            kh_sb = apool.tile([128, HG, P], bf16)
            vh_sb = apool.tile([128, HG, P], bf16)
            qt_sb = apool.tile([128, NJC, 512], bf16)

            xqv = xq.ap().rearrange("(kc p) i -> p kc i", p=128)
            xkv = xk.ap().rearrange("(kc p) i -> p kc i", p=128)

            # ---- branch phase: K, V (natural layout) and Q (transposed) ----
            for cb in range(NCB):
                cs = slice(cb * CB, (cb + 1) * CB)
                xq_t = xpool.tile([128, NKC, CB], f32r, tag="xq")
                nc.sync.dma_start(xq_t[:], xqv[:, :, cs])
                xk_t = xpool.tile([128, NKC, CB], f32r, tag="xk")
                nc.sync.dma_start(xk_t[:], xkv[:, :, cs])
                for hl in range(HG):
                    hs = slice(hl * 128, (hl + 1) * 128)
                    ps_k = pmm.tile([128, CB], f32, tag="mm")
                    for kc in range(NKC):
                        nc.tensor.matmul(ps_k[:], wk_sb[:, kc, hs], xk_t[:, kc, :],
                                         start=(kc == 0), stop=(kc == NKC - 1))
                    nc.scalar.activation(kh_sb[:, hl, cs], ps_k[:], LRELU,
                                         bias=bk_sb[:, hl:hl + 1], alpha=SLOPE)
                    ps_v = pmm.tile([128, CB], f32, tag="mm")
                    for kc in range(NKC):
                        nc.tensor.matmul(ps_v[:], wv_sb[:, kc, hs], xq_t[:, kc, :],
                                         start=(kc == 0), stop=(kc == NKC - 1))
                    nc.scalar.activation(vh_sb[:, hl, cs], ps_v[:], LRELU,
                                         bias=bv_sb[:, hl:hl + 1], alpha=SLOPE)
                for js in range(CB // 128):
                    ps_q = pmm.tile([128, 512], f32, tag="mm")
                    for kc in range(NKC):
                        nc.tensor.matmul(ps_q[:], xq_t[:, kc, js * 128:(js + 1) * 128],
                                         wq_sb[:, kc, :],
                                         start=(kc == 0), stop=False)
                    nc.tensor.matmul(ps_q[:], ones_row[:, 0:128], bq_sb[:],
                                     start=False, stop=True)
                    nc.scalar.activation(qt_sb[:, cb * (CB // 128) + js, :], ps_q[:],
                                         LRELU, alpha=SLOPE)

            # ---- attention phase, per local head and 1024-wide i-half ----
            # S^T is built 1024 wide (two 512 matmuls sharing one lhsT load)
            # so each exp eviction covers 1024 columns; the second attention
            # matmul reuses each Qt chunk for both 512-wide accumulators.
            for hl in range(HG):
                for ih in range(2):
                    pts = []
                    for jm in range(NJC):
                        ps_s = pmm.tile([128, 1024], f32, tag="mm")
                        for sub in range(2):
                            nc.tensor.matmul(ps_s[:, sub * 512:(sub + 1) * 512],
                                             vh_sb[:, hl, jm * 128:(jm + 1) * 128],
                                             kh_sb[:, hl, ih * 1024 + sub * 512:ih * 1024 + (sub + 1) * 512],
                                             start=True, stop=True)
                        pt = ptpool.tile([128, 1024], bf16, tag="pt")
                        nc.scalar.activation(pt[:], ps_s[:], EXP, scale=sc)
                        pts.append(pt)
                    ps_oa = pout.tile([128, 512], f32, tag="po")
                    ps_ob = pout.tile([128, 512], f32, tag="po")
                    for jc in range(NJC):
                        nc.tensor.matmul(ps_oa[:], qt_sb[:, jc, hl * 128:(hl + 1) * 128],
                                         pts[jc][:, 0:512], start=(jc == 0), stop=(jc == NJC - 1))
                        nc.tensor.matmul(ps_ob[:], qt_sb[:, jc, hl * 128:(hl + 1) * 128],
                                         pts[jc][:, 512:1024], start=(jc == 0), stop=(jc == NJC - 1))
                    ps_la = psm.tile([1, 512], f32, tag="sm")
                    ps_lb = psm.tile([1, 512], f32, tag="sm")
                    for jc in range(NJC):
                        nc.tensor.matmul(ps_la[:], ones_col[:], pts[jc][:, 0:512],
                                         start=(jc == 0), stop=(jc == NJC - 1))
                        nc.tensor.matmul(ps_lb[:], ones_col[:], pts[jc][:, 512:1024],
                                         start=(jc == 0), stop=(jc == NJC - 1))
                    for sub, (ps_o, ps_l) in enumerate(((ps_oa, ps_la), (ps_ob, ps_lb))):
                        lsb = opool.tile([1, 512], f32r, tag="lsb")
                        nc.scalar.activation(lsb[:], ps_l[:],
                                             mybir.ActivationFunctionType.Copy)
                        ps_b = pmm.tile([128, 512], f32, tag="mm")
                        nc.tensor.matmul(ps_b[:], ones_row[:, 0:128], lsb[:],
                                         start=True, stop=True)
                        rb = opool.tile([128, 512], f32, tag="rb")
                        nc.vector.reciprocal(rb[:], ps_b[:])
                        ot = opool.tile([128, 512], f32, tag="ot")
                        nc.vector.tensor_mul(ot[:], ps_o[:], rb[:])
                        nc.sync.dma_start(
                            out.ap()[hl * 128:(hl + 1) * 128,
                                     ih * 1024 + sub * 512:ih * 1024 + (sub + 1) * 512],
                            ot[:])

    nc.compile()
    return nc


def _get_program():
    global _PROGRAM
    if _PROGRAM is None:
        _PROGRAM = _build_program()
    return _PROGRAM


def kernel(Xq, Xk, Wq, Wk, Wv,
           gamma1, beta1, mean1, var1,
           gamma2, beta2, mean2, var2,
           gamma3, beta3, mean3, var3):
    global LAST_RESULTS
    from concourse.bass_utils import run_bass_kernel_spmd

    Xq = np.asarray(Xq, np.float32)
    Xk = np.asarray(Xk, np.float32)

    def fold(Wm, gamma, beta, mean, var):
        scale = np.asarray(gamma, np.float32) / np.sqrt(np.asarray(var, np.float32) + EPS)
        bias = np.asarray(beta, np.float32) - np.asarray(mean, np.float32) * scale
        Ws = np.asarray(Wm, np.float32) * scale[:, None]
        return Ws, bias

    Wq_s, b1 = fold(Wq, gamma1, beta1, mean1, var1)
    Wk_s, b2 = fold(Wk, gamma2, beta2, mean2, var2)
    Wv_s, b3 = fold(Wv, gamma3, beta3, mean3, var3)

    in_maps = []
    for core in range(8):
        b, hg = divmod(core, 2)
        sl = slice(hg * 512, (hg + 1) * 512)
        in_maps.append({
            "xq": np.ascontiguousarray(Xq[b]),
            "xk": np.ascontiguousarray(Xk[b]),
            "wq": np.ascontiguousarray(Wq_s[sl, :].T),
            "wk": np.ascontiguousarray(Wk_s[sl, :].T),
            "wv": np.ascontiguousarray(Wv_s[sl, :].T),
            "bq": np.ascontiguousarray(b1[sl].reshape(1, 512)),
            "ones": np.ones((1, 128), np.float32),
            "bk": np.ascontiguousarray(b2[sl].reshape(HG, 128).T),
            "bv": np.ascontiguousarray(b3[sl].reshape(HG, 128).T),
        })

    nc = _get_program()
    trace = os.environ.get("KERNEL_TRACE", "0") == "1"
    res = run_bass_kernel_spmd(nc, in_maps, core_ids=list(range(8)), trace=trace)
    LAST_RESULTS = res

    full = np.empty((B, C, P), np.float32)
    for core in range(8):
        b, hg = divmod(core, 2)
        full[b, hg * 512:(hg + 1) * 512, :] = res.results[core]["out"]
    return full


# revision 8
# speedup vs baseline: 1.2645x; 1.1627x over previous
"""Trainium2 Bass kernel for nn_CrossTransFormer_86526411145604.

Computation (b=4, C=1024, H=8 heads, dh=128, p=2048):
  Q = LeakyReLU(BN1(Wq @ Xq)), K = LeakyReLU(BN2(Wk @ Xk)), V = LeakyReLU(BN3(Wv @ Xq))
  per (b,h): S = Kh^T Vh / sqrt(dh); A = softmax_j(S); out[c,i] = sum_j A[i,j] Qh[c,j]

Sharding: 8 cores = (4 batches) x (2 head-groups of 4 heads). Each core gets
Xq[b], Xk[b] and the 512-channel slice of the (BN-scale-folded, transposed)
weights for its head group. All attention intermediates stay in SBUF; the
2048x2048 per-head softmax matrix is never materialized in HBM.

Layout trick: S is computed transposed (S^T[j,i] = Vh^T Kh) so the second
attention matmul needs no transposes: out[c,i] = sum_j Qt[j,c] P^T[j,i] with
Qt produced directly in [position, channel] layout by the Q branch
(lhsT = Xq chunk). Softmax row sums come from a ones-vector matmul; the
1/l normalization is broadcast across partitions with a rank-1 PE matmul.
"""

import math
import os

import numpy as np

C = 1024
H = 8
EPS = 1e-5
SLOPE = 0.1
B = 4
P = 2048
HG = 4            # heads per core
CB = 256          # branch column-block width
NCB = P // CB
IB = 512          # attention i-block width
NIB = P // IB
NKC = C // 128    # contraction chunks for the branch matmuls
NJC = P // 128    # j chunks for the attention contraction

_PROGRAM = None
LAST_RESULTS = None


def _build_program():
    import concourse.mybir as mybir
    import concourse.tile as tile
    from concourse import bacc

    f32 = mybir.dt.float32
    f32r = mybir.dt.float32r
    bf16 = mybir.dt.bfloat16
    LRELU = mybir.ActivationFunctionType.Prelu
    EXP = mybir.ActivationFunctionType.Exp

    nc = bacc.Bacc("TRN2", target_bir_lowering=False, debug=False)

    xq = nc.dram_tensor("xq", [C, P], f32r, kind="ExternalInput")
    xk = nc.dram_tensor("xk", [C, P], f32r, kind="ExternalInput")
    wq = nc.dram_tensor("wq", [C, 512], f32r, kind="ExternalInput")
    wk = nc.dram_tensor("wk", [C, 512], f32r, kind="ExternalInput")
    wv = nc.dram_tensor("wv", [C, 512], f32r, kind="ExternalInput")
    bq = nc.dram_tensor("bq", [1, 512], f32r, kind="ExternalInput")
    ones = nc.dram_tensor("ones", [1, 128], f32r, kind="ExternalInput")
    bk = nc.dram_tensor("bk", [128, HG], f32, kind="ExternalInput")
    bv = nc.dram_tensor("bv", [128, HG], f32, kind="ExternalInput")
    out = nc.dram_tensor("out", [512, P], f32, kind="ExternalOutput")

    sc = 1.0 / math.sqrt(C / H)

    with tile.TileContext(nc) as tc:
        with tc.tile_pool(name="wpool", bufs=1) as wpool, \
             tc.tile_pool(name="cpool", bufs=1) as cpool, \
             tc.tile_pool(name="apool", bufs=1) as apool, \
             tc.tile_pool(name="xpool", bufs=2) as xpool, \
             tc.tile_pool(name="ptpool", bufs=17) as ptpool, \
             tc.tile_pool(name="opool", bufs=2) as opool, \
             tc.tile_pool(name="pmm", bufs=2, space="PSUM") as pmm, \
             tc.tile_pool(name="pout", bufs=2, space="PSUM") as pout, \
             tc.tile_pool(name="psm", bufs=2, space="PSUM") as psm:

            wq_sb = wpool.tile([128, NKC, 512], f32r)
            nc.sync.dma_start(wq_sb[:], wq.ap().rearrange("(kc p) n -> p kc n", p=128))
            wk_sb = wpool.tile([128, NKC, 512], f32r)
            nc.sync.dma_start(wk_sb[:], wk.ap().rearrange("(kc p) n -> p kc n", p=128))
            wv_sb = wpool.tile([128, NKC, 512], f32r)
            nc.sync.dma_start(wv_sb[:], wv.ap().rearrange("(kc p) n -> p kc n", p=128))

            bq_sb = cpool.tile([1, 512], f32r)
            nc.sync.dma_start(bq_sb[:], bq.ap())
            bk_sb = cpool.tile([128, HG], f32)
            nc.sync.dma_start(bk_sb[:], bk.ap())
            bv_sb = cpool.tile([128, HG], f32)
            nc.sync.dma_start(bv_sb[:], bv.ap())

            ones_row = cpool.tile([1, 128], f32r)
            nc.sync.dma_start(ones_row[:], ones.ap())
            ones_col = cpool.tile([128, 1], bf16)
            nc.vector.memset(ones_col[:], 1.0)

            kh_sb = apool.tile([128, HG, P], bf16)
            vh_sb = apool.tile([128, HG, P], bf16)
            qt_sb = apool.tile([128, NJC, 512], bf16)

            xqv = xq.ap().rearrange("(kc p) i -> p kc i", p=128)
            xkv = xk.ap().rearrange("(kc p) i -> p kc i", p=128)

            # ---- branch phase: K, V (natural layout) and Q (transposed) ----
            for cb in range(NCB):
                cs = slice(cb * CB, (cb + 1) * CB)
                xq_t = xpool.tile([128, NKC, CB], f32r, tag="xq")
                nc.sync.dma_start(xq_t[:], xqv[:, :, cs])
                xk_t = xpool.tile([128, NKC, CB], f32r, tag="xk")
                nc.sync.dma_start(xk_t[:], xkv[:, :, cs])
                for hl in range(HG):
                    hs = slice(hl * 128, (hl + 1) * 128)
                    ps_k = pmm.tile([128, CB], f32, tag="mm")
                    for kc in range(NKC):
                        nc.tensor.matmul(ps_k[:], wk_sb[:, kc, hs], xk_t[:, kc, :],
                                         start=(kc == 0), stop=(kc == NKC - 1))
                    nc.scalar.activation(kh_sb[:, hl, cs], ps_k[:], LRELU,
                                         bias=bk_sb[:, hl:hl + 1], alpha=SLOPE)
                    ps_v = pmm.tile([128, CB], f32, tag="mm")
                    for kc in range(NKC):
                        nc.tensor.matmul(ps_v[:], wv_sb[:, kc, hs], xq_t[:, kc, :],
                                         start=(kc == 0), stop=(kc == NKC - 1))
                    nc.scalar.activation(vh_sb[:, hl, cs], ps_v[:], LRELU,
                                         bias=bv_sb[:, hl:hl + 1], alpha=SLOPE)
                for js in range(CB // 128):
                    ps_q = pmm.tile([128, 512], f32, tag="mm")
                    for kc in range(NKC):
                        nc.tensor.matmul(ps_q[:], xq_t[:, kc, js * 128:(js + 1) * 128],
                                         wq_sb[:, kc, :],
                                         start=(kc == 0), stop=False)
                    nc.tensor.matmul(ps_q[:], ones_row[:, 0:128], bq_sb[:],
                                     start=False, stop=True)
                    nc.scalar.activation(qt_sb[:, cb * (CB // 128) + js, :], ps_q[:],
                                         LRELU, alpha=SLOPE)

            # ---- attention phase, per local head and 1024-wide i-half ----
            # S^T is built 1024 wide (two 512 matmuls sharing one lhsT load)
            # so each exp eviction covers 1024 columns; the second attention
            # matmul reuses each Qt chunk for both 512-wide accumulators.
            for hl in range(HG):
                for ih in range(2):
                    pts = []
                    for jm in range(NJC):
                        ps_s = pmm.tile([128, 1024], f32, tag="mm")
                        for sub in range(2):
                            nc.tensor.matmul(ps_s[:, sub * 512:(sub + 1) * 512],
                                             vh_sb[:, hl, jm * 128:(jm + 1) * 128],
                                             kh_sb[:, hl, ih * 1024 + sub * 512:ih * 1024 + (sub + 1) * 512],
                                             start=True, stop=True)
                        pt = ptpool.tile([128, 1024], bf16, tag="pt")
                        nc.scalar.activation(pt[:], ps_s[:], EXP, scale=sc)
                        pts.append(pt)
                    ps_oa = pout.tile([128, 512], f32, tag="po")
                    ps_ob = pout.tile([128, 512], f32, tag="po")
                    ps_la = psm.tile([1, 512], f32, tag="sm")
                    ps_lb = psm.tile([1, 512], f32, tag="sm")
                    for jc in range(NJC):
                        nc.tensor.matmul(ps_oa[:], qt_sb[:, jc, hl * 128:(hl + 1) * 128],
                                         pts[jc][:, 0:512], start=(jc == 0), stop=(jc == NJC - 1))
                        nc.tensor.matmul(ps_ob[:], qt_sb[:, jc, hl * 128:(hl + 1) * 128],
                                         pts[jc][:, 512:1024], start=(jc == 0), stop=(jc == NJC - 1))
                        nc.tensor.matmul(ps_la[:], ones_col[:], pts[jc][:, 0:512],
                                         start=(jc == 0), stop=(jc == NJC - 1))
                        nc.tensor.matmul(ps_lb[:], ones_col[:], pts[jc][:, 512:1024],
                                         start=(jc == 0), stop=(jc == NJC - 1))
                    for sub, (ps_o, ps_l) in enumerate(((ps_oa, ps_la), (ps_ob, ps_lb))):
                        rb1 = opool.tile([1, 512], f32, tag="rb1")
                        nc.vector.reciprocal_approx_fast(rb1[:], ps_l[:])
                        rbc = opool.tile([128, 512], f32, tag="rbc")
                        nc.gpsimd.partition_broadcast(rbc[:], rb1[:])
                        ot = opool.tile([128, 512], f32, tag="ot")
                        nc.vector.tensor_mul(ot[:], ps_o[:], rbc[:])
                        nc.sync.dma_start(
                            out.ap()[hl * 128:(hl + 1) * 128,
                                     ih * 1024 + sub * 512:ih * 1024 + (sub + 1) * 512],
                            ot[:])

    nc.compile()
    return nc


def _get_program():
    global _PROGRAM
    if _PROGRAM is None:
        _PROGRAM = _build_program()
    return _PROGRAM


def kernel(Xq, Xk, Wq, Wk, Wv,
           gamma1, beta1, mean1, var1,
           gamma2, beta2, mean2, var2,
           gamma3, beta3, mean3, var3):
    global LAST_RESULTS
    from concourse.bass_utils import run_bass_kernel_spmd

    Xq = np.asarray(Xq, np.float32)
    Xk = np.asarray(Xk, np.float32)

    def fold(Wm, gamma, beta, mean, var):
        scale = np.asarray(gamma, np.float32) / np.sqrt(np.asarray(var, np.float32) + EPS)
        bias = np.asarray(beta, np.float32) - np.asarray(mean, np.float32) * scale
        Ws = np.asarray(Wm, np.float32) * scale[:, None]
        return Ws, bias

    Wq_s, b1 = fold(Wq, gamma1, beta1, mean1, var1)
    Wk_s, b2 = fold(Wk, gamma2, beta2, mean2, var2)
    Wv_s, b3 = fold(Wv, gamma3, beta3, mean3, var3)

    in_maps = []
    for core in range(8):
        b, hg = divmod(core, 2)
        sl = slice(hg * 512, (hg + 1) * 512)
        in_maps.append({
            "xq": np.ascontiguousarray(Xq[b]),
            "xk": np.ascontiguousarray(Xk[b]),
            "wq": np.ascontiguousarray(Wq_s[sl, :].T),
            "wk": np.ascontiguousarray(Wk_s[sl, :].T),
            "wv": np.ascontiguousarray(Wv_s[sl, :].T),
            "bq": np.ascontiguousarray(b1[sl].reshape(1, 512)),
            "ones": np.ones((1, 128), np.float32),
            "bk": np.ascontiguousarray(b2[sl].reshape(HG, 128).T),
            "bv": np.ascontiguousarray(b3[sl].reshape(HG, 128).T),
        })

    nc = _get_program()
    trace = os.environ.get("KERNEL_TRACE", "0") == "1"
    res = run_bass_kernel_spmd(nc, in_maps, core_ids=list(range(8)), trace=trace)
    LAST_RESULTS = res

    full = np.empty((B, C, P), np.float32)
    for core in range(8):
        b, hg = divmod(core, 2)
        full[b, hg * 512:(hg + 1) * 512, :] = res.results[core]["out"]
    return full


# revision 11
# speedup vs baseline: 1.2661x; 1.0012x over previous
"""Trainium2 Bass kernel for nn_CrossTransFormer_86526411145604.

Computation (b=4, C=1024, H=8 heads, dh=128, p=2048):
  Q = LeakyReLU(BN1(Wq @ Xq)), K = LeakyReLU(BN2(Wk @ Xk)), V = LeakyReLU(BN3(Wv @ Xq))
  per (b,h): S = Kh^T Vh / sqrt(dh); A = softmax_j(S); out[c,i] = sum_j A[i,j] Qh[c,j]

Sharding: 8 cores = (4 batches) x (2 head-groups of 4 heads). Each core gets
Xq[b], Xk[b] and the 512-channel slice of the (BN-scale-folded, transposed)
weights for its head group. All attention intermediates stay in SBUF; the
2048x2048 per-head softmax matrix is never materialized in HBM.

Layout trick: S is computed transposed (S^T[j,i] = Vh^T Kh) so the second
attention matmul needs no transposes: out[c,i] = sum_j Qt[j,c] P^T[j,i] with
Qt produced directly in [position, channel] layout by the Q branch
(lhsT = Xq chunk). Softmax row sums come from a ones-vector matmul; the
1/l normalization is broadcast across partitions with a rank-1 PE matmul.
"""

import math
import os

import numpy as np

C = 1024
H = 8
EPS = 1e-5
SLOPE = 0.1
B = 4
P = 2048
HG = 4            # heads per core
CB = 256          # branch column-block width
NCB = P // CB
IB = 512          # attention i-block width
NIB = P // IB
NKC = C // 128    # contraction chunks for the branch matmuls
NJC = P // 128    # j chunks for the attention contraction

_PROGRAM = None
LAST_RESULTS = None


def _patch_ldw_opt():
    # walrus's LDWEIGHTS merge is off by default in this harness; enabling it
    # hides the per-matmul weight-load issue cost.
    import concourse.bass_utils as bu
    if getattr(bu, "_ldw_patched", False):
        return
    orig = bu.run_command

    def patched(argv, **kw):
        argv = ["--enable-ldw-opt=true" if a == "--enable-ldw-opt=false" else a
                for a in argv]
        return orig(argv, **kw)

    bu.run_command = patched
    bu._ldw_patched = True


def _build_program():
    import concourse.mybir as mybir
    import concourse.tile as tile
    from concourse import bacc

    if os.environ.get("LDW_OPT", "0") == "1":
        _patch_ldw_opt()

    f32 = mybir.dt.float32
    f32r = mybir.dt.float32r
    bf16 = mybir.dt.bfloat16
    LRELU = mybir.ActivationFunctionType.Prelu
    EXP = mybir.ActivationFunctionType.Exp

    nc = bacc.Bacc("TRN2", target_bir_lowering=False, debug=False)

    xq = nc.dram_tensor("xq", [C, P], f32r, kind="ExternalInput")
    xk = nc.dram_tensor("xk", [C, P], f32r, kind="ExternalInput")
    wq = nc.dram_tensor("wq", [C, 512], f32r, kind="ExternalInput")
    wk = nc.dram_tensor("wk", [C, 512], f32r, kind="ExternalInput")
    wv = nc.dram_tensor("wv", [C, 512], f32r, kind="ExternalInput")
    bq = nc.dram_tensor("bq", [1, 512], f32r, kind="ExternalInput")
    ones = nc.dram_tensor("ones", [1, 128], f32r, kind="ExternalInput")
    bk = nc.dram_tensor("bk", [128, HG], f32, kind="ExternalInput")
    bv = nc.dram_tensor("bv", [128, HG], f32, kind="ExternalInput")
    out = nc.dram_tensor("out", [512, P], f32, kind="ExternalOutput")

    sc = 1.0 / math.sqrt(C / H)

    with tile.TileContext(nc) as tc:
        with tc.tile_pool(name="wpool", bufs=1) as wpool, \
             tc.tile_pool(name="cpool", bufs=1) as cpool, \
             tc.tile_pool(name="apool", bufs=1) as apool, \
             tc.tile_pool(name="xpool", bufs=2) as xpool, \
             tc.tile_pool(name="ptpool", bufs=18) as ptpool, \
             tc.tile_pool(name="opool", bufs=2) as opool, \
             tc.tile_pool(name="pmm", bufs=2, space="PSUM") as pmm, \
             tc.tile_pool(name="pout", bufs=2, space="PSUM") as pout, \
             tc.tile_pool(name="psm", bufs=2, space="PSUM") as psm:

            wk_sb = wpool.tile([128, NKC, 512], f32r)
            wv_sb = wpool.tile([128, NKC, 512], f32r)
            wq_sb = wpool.tile([128, NKC, 512], f32r)
            # K-branch runs first: land wk (then wv/wq) in half-tensor chunks
            # so the first accumulation group starts long before all weights
            # arrive.
            for wsb, wdr in ((wk_sb, wk), (wv_sb, wv), (wq_sb, wq)):
                wview = wdr.ap().rearrange("(kc p) n -> p kc n", p=128)
                for half in range(2):
                    hs4 = slice(half * NKC // 2, (half + 1) * NKC // 2)
                    nc.sync.dma_start(wsb[:, hs4, :], wview[:, hs4, :])

            bq_sb = cpool.tile([1, 512], f32r)
            nc.sync.dma_start(bq_sb[:], bq.ap())
            bk_sb = cpool.tile([128, HG], f32)
            nc.sync.dma_start(bk_sb[:], bk.ap())
            bv_sb = cpool.tile([128, HG], f32)
            nc.sync.dma_start(bv_sb[:], bv.ap())

            ones_row = cpool.tile([1, 128], f32r)
            nc.sync.dma_start(ones_row[:], ones.ap())
            ones_col = cpool.tile([128, 1], bf16)
            nc.vector.memset(ones_col[:], 1.0)

            kh_sb = apool.tile([128, HG, P], bf16)
            vh_sb = apool.tile([128, HG, P], bf16)
            qt_sb = apool.tile([128, NJC, 512], bf16)

            xqv = xq.ap().rearrange("(kc p) i -> p kc i", p=128)
            xkv = xk.ap().rearrange("(kc p) i -> p kc i", p=128)

            # ---- branch phase: K, V (natural layout) and Q (transposed) ----
            for cb in range(NCB):
                cs = slice(cb * CB, (cb + 1) * CB)
                xk_t = xpool.tile([128, NKC, CB], f32r, tag="xk")
                for half in range(2):
                    hs4 = slice(half * NKC // 2, (half + 1) * NKC // 2)
                    nc.sync.dma_start(xk_t[:, hs4, :], xkv[:, hs4, cs])
                xq_t = xpool.tile([128, NKC, CB], f32r, tag="xq")
                for half in range(2):
                    hs4 = slice(half * NKC // 2, (half + 1) * NKC // 2)
                    nc.sync.dma_start(xq_t[:, hs4, :], xqv[:, hs4, cs])
                for hl in range(HG):
                    hs = slice(hl * 128, (hl + 1) * 128)
                    ps_k = pmm.tile([128, CB], f32, tag="mm")
                    for kc in range(NKC):
                        nc.tensor.matmul(ps_k[:], wk_sb[:, kc, hs], xk_t[:, kc, :],
                                         start=(kc == 0), stop=(kc == NKC - 1))
                    nc.scalar.activation(kh_sb[:, hl, cs], ps_k[:], LRELU,
                                         bias=bk_sb[:, hl:hl + 1], alpha=SLOPE)
                    ps_v = pmm.tile([128, CB], f32, tag="mm")
                    for kc in range(NKC):
                        nc.tensor.matmul(ps_v[:], wv_sb[:, kc, hs], xq_t[:, kc, :],
                                         start=(kc == 0), stop=(kc == NKC - 1))
                    nc.scalar.activation(vh_sb[:, hl, cs], ps_v[:], LRELU,
                                         bias=bv_sb[:, hl:hl + 1], alpha=SLOPE)
                for js in range(CB // 128):
                    ps_q = pmm.tile([128, 512], f32, tag="mm")
                    for kc in range(NKC):
                        nc.tensor.matmul(ps_q[:], xq_t[:, kc, js * 128:(js + 1) * 128],
                                         wq_sb[:, kc, :],
                                         start=(kc == 0), stop=False)
                    nc.tensor.matmul(ps_q[:], ones_row[:, 0:128], bq_sb[:],
                                     start=False, stop=True)
                    nc.scalar.activation(qt_sb[:, cb * (CB // 128) + js, :], ps_q[:],
                                         LRELU, alpha=SLOPE)

            # ---- attention phase, per local head and 1024-wide i-half ----
            # S^T is built 1024 wide (two 512 matmuls sharing one lhsT load)
            # so each exp eviction covers 1024 columns; the second attention
            # matmul reuses each Qt chunk for both 512-wide accumulators.
            for hl in range(HG):
                for ih in range(2):
                    pts = []
                    for jm in range(NJC):
                        ps_s = pmm.tile([128, 1024], f32, tag="mm")
                        for sub in range(2):
                            nc.tensor.matmul(ps_s[:, sub * 512:(sub + 1) * 512],
                                             vh_sb[:, hl, jm * 128:(jm + 1) * 128],
                                             kh_sb[:, hl, ih * 1024 + sub * 512:ih * 1024 + (sub + 1) * 512],
                                             start=True, stop=True)
                        pt = ptpool.tile([128, 1024], bf16, tag="pt")
                        nc.scalar.activation(pt[:], ps_s[:], EXP, scale=sc)
                        pts.append(pt)
                    ps_oa = pout.tile([128, 512], f32, tag="po")
                    ps_ob = pout.tile([128, 512], f32, tag="po")
                    ps_la = psm.tile([1, 512], f32, tag="sm")
                    ps_lb = psm.tile([1, 512], f32, tag="sm")
                    for jc in range(NJC):
                        nc.tensor.matmul(ps_oa[:], qt_sb[:, jc, hl * 128:(hl + 1) * 128],
                                         pts[jc][:, 0:512], start=(jc == 0), stop=(jc == NJC - 1))
                        nc.tensor.matmul(ps_ob[:], qt_sb[:, jc, hl * 128:(hl + 1) * 128],
                                         pts[jc][:, 512:1024], start=(jc == 0), stop=(jc == NJC - 1))
                        nc.tensor.matmul(ps_la[:], ones_col[:], pts[jc][:, 0:512],
                                         start=(jc == 0), stop=(jc == NJC - 1))
                        nc.tensor.matmul(ps_lb[:], ones_col[:], pts[jc][:, 512:1024],
                                         start=(jc == 0), stop=(jc == NJC - 1))
                    for sub, (ps_o, ps_l) in enumerate(((ps_oa, ps_la), (ps_ob, ps_lb))):
                        rb1 = opool.tile([1, 512], f32, tag="rb1")
                        nc.vector.reciprocal_approx_fast(rb1[:], ps_l[:])
                        rbc = opool.tile([128, 512], f32, tag="rbc")
                        nc.gpsimd.partition_broadcast(rbc[:], rb1[:])
                        ot = opool.tile([128, 512], f32, tag="ot")
                        nc.vector.tensor_mul(ot[:], ps_o[:], rbc[:])
                        nc.sync.dma_start(
                            out.ap()[hl * 128:(hl + 1) * 128,
                                     ih * 1024 + sub * 512:ih * 1024 + (sub + 1) * 512],
                            ot[:])

    nc.compile()
    return nc


def _get_program():
    global _PROGRAM
    if _PROGRAM is None:
        _PROGRAM = _build_program()
    return _PROGRAM


def kernel(Xq, Xk, Wq, Wk, Wv,
           gamma1, beta1, mean1, var1,
           gamma2, beta2, mean2, var2,
           gamma3, beta3, mean3, var3):
    global LAST_RESULTS
    from concourse.bass_utils import run_bass_kernel_spmd

    Xq = np.asarray(Xq, np.float32)
    Xk = np.asarray(Xk, np.float32)

    def fold(Wm, gamma, beta, mean, var):
        scale = np.asarray(gamma, np.float32) / np.sqrt(np.asarray(var, np.float32) + EPS)
        bias = np.asarray(beta, np.float32) - np.asarray(mean, np.float32) * scale
        Ws = np.asarray(Wm, np.float32) * scale[:, None]
        return Ws, bias

    Wq_s, b1 = fold(Wq, gamma1, beta1, mean1, var1)
    Wk_s, b2 = fold(Wk, gamma2, beta2, mean2, var2)
    Wv_s, b3 = fold(Wv, gamma3, beta3, mean3, var3)

    in_maps = []
    for core in range(8):
        b, hg = divmod(core, 2)
        sl = slice(hg * 512, (hg + 1) * 512)
        in_maps.append({
            "xq": np.ascontiguousarray(Xq[b]),
            "xk": np.ascontiguousarray(Xk[b]),
            "wq": np.ascontiguousarray(Wq_s[sl, :].T),
            "wk": np.ascontiguousarray(Wk_s[sl, :].T),
            "wv": np.ascontiguousarray(Wv_s[sl, :].T),
            "bq": np.ascontiguousarray(b1[sl].reshape(1, 512)),
            "ones": np.ones((1, 128), np.float32),
            "bk": np.ascontiguousarray(b2[sl].reshape(HG, 128).T),
            "bv": np.ascontiguousarray(b3[sl].reshape(HG, 128).T),
        })

    nc = _get_program()
    trace = os.environ.get("KERNEL_TRACE", "0") == "1"
    res = run_bass_kernel_spmd(nc, in_maps, core_ids=list(range(8)), trace=trace)
    LAST_RESULTS = res

    full = np.empty((B, C, P), np.float32)
    for core in range(8):
        b, hg = divmod(core, 2)
        full[b, hg * 512:(hg + 1) * 512, :] = res.results[core]["out"]
    return full


# revision 12
# speedup vs baseline: 1.3523x; 1.0681x over previous
"""Trainium2 Bass kernel for nn_CrossTransFormer_86526411145604.

Computation (b=4, C=1024, H=8 heads, dh=128, p=2048):
  Q = LeakyReLU(BN1(Wq @ Xq)), K = LeakyReLU(BN2(Wk @ Xk)), V = LeakyReLU(BN3(Wv @ Xq))
  per (b,h): S = Kh^T Vh / sqrt(dh); A = softmax_j(S); out[c,i] = sum_j A[i,j] Qh[c,j]

Sharding: 8 cores = (4 batches) x (2 head-groups of 4 heads). Each core gets
Xq[b], Xk[b] and the 512-channel slice of the (BN-scale-folded, transposed)
weights for its head group. All attention intermediates stay in SBUF; the
2048x2048 per-head softmax matrix is never materialized in HBM.

Layout trick: S is computed transposed (S^T[j,i] = Vh^T Kh) so the second
attention matmul needs no transposes: out[c,i] = sum_j Qt[j,c] P^T[j,i] with
Qt produced directly in [position, channel] layout by the Q branch
(lhsT = Xq chunk). Softmax row sums come from a ones-vector matmul; the
1/l normalization is broadcast across partitions with a rank-1 PE matmul.
"""

import math
import os

import numpy as np

C = 1024
H = 8
EPS = 1e-5
SLOPE = 0.1
B = 4
P = 2048
HG = 4            # heads per core
CB = 256          # branch column-block width
NCB = P // CB
IB = 512          # attention i-block width
NIB = P // IB
NKC = C // 128    # contraction chunks for the branch matmuls
NJC = P // 128    # j chunks for the attention contraction

_PROGRAM = None
LAST_RESULTS = None


def _patch_ldw_opt():
    # walrus's LDWEIGHTS merge is off by default in this harness; enabling it
    # hides the per-matmul weight-load issue cost.
    import concourse.bass_utils as bu
    if getattr(bu, "_ldw_patched", False):
        return
    orig = bu.run_command

    def patched(argv, **kw):
        argv = ["--enable-ldw-opt=true" if a == "--enable-ldw-opt=false" else a
                for a in argv]
        return orig(argv, **kw)

    bu.run_command = patched
    bu._ldw_patched = True


def _build_program():
    import concourse.mybir as mybir
    import concourse.tile as tile
    from concourse import bacc

    if os.environ.get("LDW_OPT", "0") == "1":
        _patch_ldw_opt()

    f32 = mybir.dt.float32
    f32r = mybir.dt.float32r
    bf16 = mybir.dt.bfloat16
    LRELU = mybir.ActivationFunctionType.Prelu
    EXP = mybir.ActivationFunctionType.Exp

    nc = bacc.Bacc("TRN2", target_bir_lowering=False, debug=False)

    xq = nc.dram_tensor("xq", [C, P], bf16, kind="ExternalInput")
    xk = nc.dram_tensor("xk", [C, P], bf16, kind="ExternalInput")
    wq = nc.dram_tensor("wq", [C, 512], bf16, kind="ExternalInput")
    wk = nc.dram_tensor("wk", [C, 512], bf16, kind="ExternalInput")
    wv = nc.dram_tensor("wv", [C, 512], bf16, kind="ExternalInput")
    bq = nc.dram_tensor("bq", [1, 512], bf16, kind="ExternalInput")
    ones = nc.dram_tensor("ones", [1, 128], bf16, kind="ExternalInput")
    bk = nc.dram_tensor("bk", [128, HG], f32, kind="ExternalInput")
    bv = nc.dram_tensor("bv", [128, HG], f32, kind="ExternalInput")
    out = nc.dram_tensor("out", [512, P], f32, kind="ExternalOutput")

    sc = 1.0 / math.sqrt(C / H)

    with tile.TileContext(nc) as tc:
        with tc.tile_pool(name="wpool", bufs=1) as wpool, \
             tc.tile_pool(name="cpool", bufs=1) as cpool, \
             tc.tile_pool(name="apool", bufs=1) as apool, \
             tc.tile_pool(name="xpool", bufs=2) as xpool, \
             tc.tile_pool(name="ptpool", bufs=18) as ptpool, \
             tc.tile_pool(name="opool", bufs=2) as opool, \
             tc.tile_pool(name="pmm", bufs=2, space="PSUM") as pmm, \
             tc.tile_pool(name="pout", bufs=2, space="PSUM") as pout, \
             tc.tile_pool(name="psm", bufs=2, space="PSUM") as psm:

            wk_sb = wpool.tile([128, NKC, 512], bf16)
            wv_sb = wpool.tile([128, NKC, 512], bf16)
            wq_sb = wpool.tile([128, NKC, 512], bf16)
            # K-branch runs first: land wk (then wv/wq) in half-tensor chunks
            # so the first accumulation group starts long before all weights
            # arrive.
            for wsb, wdr in ((wk_sb, wk), (wv_sb, wv), (wq_sb, wq)):
                wview = wdr.ap().rearrange("(kc p) n -> p kc n", p=128)
                for half in range(2):
                    hs4 = slice(half * NKC // 2, (half + 1) * NKC // 2)
                    nc.sync.dma_start(wsb[:, hs4, :], wview[:, hs4, :])

            bq_sb = cpool.tile([1, 512], bf16)
            nc.sync.dma_start(bq_sb[:], bq.ap())
            bk_sb = cpool.tile([128, HG], f32)
            nc.sync.dma_start(bk_sb[:], bk.ap())
            bv_sb = cpool.tile([128, HG], f32)
            nc.sync.dma_start(bv_sb[:], bv.ap())

            ones_row = cpool.tile([1, 128], bf16)
            nc.sync.dma_start(ones_row[:], ones.ap())
            ones_col = cpool.tile([128, 1], bf16)
            nc.vector.memset(ones_col[:], 1.0)

            kh_sb = apool.tile([128, HG, P], bf16)
            vh_sb = apool.tile([128, HG, P], bf16)
            qt_sb = apool.tile([128, NJC, 512], bf16)

            xqv = xq.ap().rearrange("(kc p) i -> p kc i", p=128)
            xkv = xk.ap().rearrange("(kc p) i -> p kc i", p=128)

            # ---- branch phase: K, V (natural layout) and Q (transposed) ----
            for cb in range(NCB):
                cs = slice(cb * CB, (cb + 1) * CB)
                xk_t = xpool.tile([128, NKC, CB], bf16, tag="xk")
                for half in range(2):
                    hs4 = slice(half * NKC // 2, (half + 1) * NKC // 2)
                    nc.sync.dma_start(xk_t[:, hs4, :], xkv[:, hs4, cs])
                xq_t = xpool.tile([128, NKC, CB], bf16, tag="xq")
                for half in range(2):
                    hs4 = slice(half * NKC // 2, (half + 1) * NKC // 2)
                    nc.sync.dma_start(xq_t[:, hs4, :], xqv[:, hs4, cs])
                for hl in range(HG):
                    hs = slice(hl * 128, (hl + 1) * 128)
                    ps_k = pmm.tile([128, CB], f32, tag="mm")
                    for kc in range(NKC):
                        nc.tensor.matmul(ps_k[:], wk_sb[:, kc, hs], xk_t[:, kc, :],
                                         start=(kc == 0), stop=(kc == NKC - 1))
                    nc.scalar.activation(kh_sb[:, hl, cs], ps_k[:], LRELU,
                                         bias=bk_sb[:, hl:hl + 1], alpha=SLOPE)
                    ps_v = pmm.tile([128, CB], f32, tag="mm")
                    for kc in range(NKC):
                        nc.tensor.matmul(ps_v[:], wv_sb[:, kc, hs], xq_t[:, kc, :],
                                         start=(kc == 0), stop=(kc == NKC - 1))
                    nc.scalar.activation(vh_sb[:, hl, cs], ps_v[:], LRELU,
                                         bias=bv_sb[:, hl:hl + 1], alpha=SLOPE)
                for js in range(CB // 128):
                    ps_q = pmm.tile([128, 512], f32, tag="mm")
                    for kc in range(NKC):
                        nc.tensor.matmul(ps_q[:], xq_t[:, kc, js * 128:(js + 1) * 128],
                                         wq_sb[:, kc, :],
                                         start=(kc == 0), stop=False)
                    nc.tensor.matmul(ps_q[:], ones_row[:, 0:128], bq_sb[:],
                                     start=False, stop=True)
                    nc.scalar.activation(qt_sb[:, cb * (CB // 128) + js, :], ps_q[:],
                                         LRELU, alpha=SLOPE)

            # ---- attention phase, per local head and 1024-wide i-half ----
            # S^T is built 1024 wide (two 512 matmuls sharing one lhsT load)
            # so each exp eviction covers 1024 columns; the second attention
            # matmul reuses each Qt chunk for both 512-wide accumulators.
            for hl in range(HG):
                for ih in range(2):
                    pts = []
                    for jm in range(NJC):
                        ps_s = pmm.tile([128, 1024], f32, tag="mm")
                        for sub in range(2):
                            nc.tensor.matmul(ps_s[:, sub * 512:(sub + 1) * 512],
                                             vh_sb[:, hl, jm * 128:(jm + 1) * 128],
                                             kh_sb[:, hl, ih * 1024 + sub * 512:ih * 1024 + (sub + 1) * 512],
                                             start=True, stop=True)
                        pt = ptpool.tile([128, 1024], bf16, tag="pt")
                        nc.scalar.activation(pt[:], ps_s[:], EXP, scale=sc)
                        pts.append(pt)
                    ps_oa = pout.tile([128, 512], f32, tag="po")
                    ps_ob = pout.tile([128, 512], f32, tag="po")
                    ps_la = psm.tile([1, 512], f32, tag="sm")
                    ps_lb = psm.tile([1, 512], f32, tag="sm")
                    for jc in range(NJC):
                        nc.tensor.matmul(ps_oa[:], qt_sb[:, jc, hl * 128:(hl + 1) * 128],
                                         pts[jc][:, 0:512], start=(jc == 0), stop=(jc == NJC - 1))
                        nc.tensor.matmul(ps_ob[:], qt_sb[:, jc, hl * 128:(hl + 1) * 128],
                                         pts[jc][:, 512:1024], start=(jc == 0), stop=(jc == NJC - 1))
                        nc.tensor.matmul(ps_la[:], ones_col[:], pts[jc][:, 0:512],
                                         start=(jc == 0), stop=(jc == NJC - 1))
                        nc.tensor.matmul(ps_lb[:], ones_col[:], pts[jc][:, 512:1024],
                                         start=(jc == 0), stop=(jc == NJC - 1))
                    for sub, (ps_o, ps_l) in enumerate(((ps_oa, ps_la), (ps_ob, ps_lb))):
                        rb1 = opool.tile([1, 512], f32, tag="rb1")
                        nc.vector.reciprocal_approx_fast(rb1[:], ps_l[:])
                        rbc = opool.tile([128, 512], f32, tag="rbc")
                        nc.gpsimd.partition_broadcast(rbc[:], rb1[:])
                        ot = opool.tile([128, 512], f32, tag="ot")
                        nc.vector.tensor_mul(ot[:], ps_o[:], rbc[:])
                        nc.sync.dma_start(
                            out.ap()[hl * 128:(hl + 1) * 128,
                                     ih * 1024 + sub * 512:ih * 1024 + (sub + 1) * 512],
                            ot[:])

    nc.compile()
    return nc


def _get_program():
    global _PROGRAM
    if _PROGRAM is None:
        _PROGRAM = _build_program()
    return _PROGRAM


def kernel(Xq, Xk, Wq, Wk, Wv,
           gamma1, beta1, mean1, var1,
           gamma2, beta2, mean2, var2,
           gamma3, beta3, mean3, var3):
    global LAST_RESULTS
    from concourse.bass_utils import run_bass_kernel_spmd

    Xq = np.asarray(Xq, np.float32)
    Xk = np.asarray(Xk, np.float32)

    def fold(Wm, gamma, beta, mean, var):
        scale = np.asarray(gamma, np.float32) / np.sqrt(np.asarray(var, np.float32) + EPS)
        bias = np.asarray(beta, np.float32) - np.asarray(mean, np.float32) * scale
        Ws = np.asarray(Wm, np.float32) * scale[:, None]
        return Ws, bias

    Wq_s, b1 = fold(Wq, gamma1, beta1, mean1, var1)
    Wk_s, b2 = fold(Wk, gamma2, beta2, mean2, var2)
    Wv_s, b3 = fold(Wv, gamma3, beta3, mean3, var3)

    import ml_dtypes
    bf = ml_dtypes.bfloat16
    Xq_b = Xq.astype(bf)
    Xk_b = Xk.astype(bf)
    in_maps = []
    for core in range(8):
        b, hg = divmod(core, 2)
        sl = slice(hg * 512, (hg + 1) * 512)
        in_maps.append({
            "xq": np.ascontiguousarray(Xq_b[b]),
            "xk": np.ascontiguousarray(Xk_b[b]),
            "wq": np.ascontiguousarray(Wq_s[sl, :].T.astype(bf)),
            "wk": np.ascontiguousarray(Wk_s[sl, :].T.astype(bf)),
            "wv": np.ascontiguousarray(Wv_s[sl, :].T.astype(bf)),
            "bq": np.ascontiguousarray(b1[sl].reshape(1, 512).astype(bf)),
            "ones": np.ones((1, 128), bf),
            "bk": np.ascontiguousarray(b2[sl].reshape(HG, 128).T),
            "bv": np.ascontiguousarray(b3[sl].reshape(HG, 128).T),
        })

    nc = _get_program()
    trace = os.environ.get("KERNEL_TRACE", "0") == "1"
    res = run_bass_kernel_spmd(nc, in_maps, core_ids=list(range(8)), trace=trace)
    LAST_RESULTS = res

    full = np.empty((B, C, P), np.float32)
    for core in range(8):
        b, hg = divmod(core, 2)
        full[b, hg * 512:(hg + 1) * 512, :] = res.results[core]["out"]
    return full


# revision 13
# speedup vs baseline: 1.3684x; 1.0119x over previous
"""Trainium2 Bass kernel for nn_CrossTransFormer_86526411145604.

Computation (b=4, C=1024, H=8 heads, dh=128, p=2048):
  Q = LeakyReLU(BN1(Wq @ Xq)), K = LeakyReLU(BN2(Wk @ Xk)), V = LeakyReLU(BN3(Wv @ Xq))
  per (b,h): S = Kh^T Vh / sqrt(dh); A = softmax_j(S); out[c,i] = sum_j A[i,j] Qh[c,j]

Sharding: 8 cores = (4 batches) x (2 head-groups of 4 heads). Each core gets
Xq[b], Xk[b] and the 512-channel slice of the (BN-scale-folded, transposed)
weights for its head group. All attention intermediates stay in SBUF; the
2048x2048 per-head softmax matrix is never materialized in HBM.

Layout trick: S is computed transposed (S^T[j,i] = Vh^T Kh) so the second
attention matmul needs no transposes: out[c,i] = sum_j Qt[j,c] P^T[j,i] with
Qt produced directly in [position, channel] layout by the Q branch
(lhsT = Xq chunk). Softmax row sums come from a ones-vector matmul; the
1/l normalization is broadcast across partitions with a rank-1 PE matmul.
"""

import math
import os

import numpy as np

C = 1024
H = 8
EPS = 1e-5
SLOPE = 0.1
B = 4
P = 2048
HG = 4            # heads per core
CB = 256          # branch column-block width
NCB = P // CB
IB = 512          # attention i-block width
NIB = P // IB
NKC = C // 128    # contraction chunks for the branch matmuls
NJC = P // 128    # j chunks for the attention contraction

_PROGRAM = None
LAST_RESULTS = None


def _patch_ldw_opt():
    # walrus's LDWEIGHTS merge is off by default in this harness; enabling it
    # hides the per-matmul weight-load issue cost.
    import concourse.bass_utils as bu
    if getattr(bu, "_ldw_patched", False):
        return
    orig = bu.run_command

    def patched(argv, **kw):
        argv = ["--enable-ldw-opt=true" if a == "--enable-ldw-opt=false" else a
                for a in argv]
        return orig(argv, **kw)

    bu.run_command = patched
    bu._ldw_patched = True


def _build_program():
    import concourse.mybir as mybir
    import concourse.tile as tile
    from concourse import bacc

    if os.environ.get("LDW_OPT", "0") == "1":
        _patch_ldw_opt()

    f32 = mybir.dt.float32
    f32r = mybir.dt.float32r
    bf16 = mybir.dt.bfloat16
    LRELU = mybir.ActivationFunctionType.Prelu
    EXP = mybir.ActivationFunctionType.Exp

    nc = bacc.Bacc("TRN2", target_bir_lowering=False, debug=False)

    xq = nc.dram_tensor("xq", [C, P], bf16, kind="ExternalInput")
    xk = nc.dram_tensor("xk", [C, P], bf16, kind="ExternalInput")
    wq = nc.dram_tensor("wq", [C, 512], bf16, kind="ExternalInput")
    wk = nc.dram_tensor("wk", [C, 512], bf16, kind="ExternalInput")
    wv = nc.dram_tensor("wv", [C, 512], bf16, kind="ExternalInput")
    bq = nc.dram_tensor("bq", [1, 512], bf16, kind="ExternalInput")
    ones = nc.dram_tensor("ones", [1, 128], bf16, kind="ExternalInput")
    bk = nc.dram_tensor("bk", [128, HG], f32, kind="ExternalInput")
    bv = nc.dram_tensor("bv", [128, HG], f32, kind="ExternalInput")
    out = nc.dram_tensor("out", [512, P], f32, kind="ExternalOutput")

    sc = 1.0 / math.sqrt(C / H)

    with tile.TileContext(nc) as tc:
        with tc.tile_pool(name="wpool", bufs=1) as wpool, \
             tc.tile_pool(name="cpool", bufs=1) as cpool, \
             tc.tile_pool(name="apool", bufs=1) as apool, \
             tc.tile_pool(name="xpool", bufs=2) as xpool, \
             tc.tile_pool(name="ptpool", bufs=18) as ptpool, \
             tc.tile_pool(name="opool", bufs=2) as opool, \
             tc.tile_pool(name="pmm", bufs=2, space="PSUM") as pmm, \
             tc.tile_pool(name="pout", bufs=2, space="PSUM") as pout, \
             tc.tile_pool(name="psm", bufs=2, space="PSUM") as psm:

            wk_sb = wpool.tile([128, NKC, 512], bf16)
            wv_sb = wpool.tile([128, NKC, 512], bf16)
            wq_sb = wpool.tile([128, NKC, 512], bf16)
            # K-branch runs first: issue only wk + its bias up front; the
            # remaining weight/bias loads are issued after the first
            # col-block's X tiles so the first accumulation group starts as
            # early as possible.
            def _load_w(wsb, wdr):
                wview = wdr.ap().rearrange("(kc p) n -> p kc n", p=128)
                for half in range(2):
                    hs4 = slice(half * NKC // 2, (half + 1) * NKC // 2)
                    nc.sync.dma_start(wsb[:, hs4, :], wview[:, hs4, :])

            _load_w(wk_sb, wk)
            bk_sb = cpool.tile([128, HG], f32)
            nc.sync.dma_start(bk_sb[:], bk.ap())
            bq_sb = cpool.tile([1, 512], bf16)
            bv_sb = cpool.tile([128, HG], f32)
            ones_row = cpool.tile([1, 128], bf16)
            ones_col = cpool.tile([128, 1], bf16)
            nc.vector.memset(ones_col[:], 1.0)

            kh_sb = apool.tile([128, HG, P], bf16)
            vh_sb = apool.tile([128, HG, P], bf16)
            qt_sb = apool.tile([128, NJC, 512], bf16)

            xqv = xq.ap().rearrange("(kc p) i -> p kc i", p=128)
            xkv = xk.ap().rearrange("(kc p) i -> p kc i", p=128)

            # ---- branch phase: K, V (natural layout) and Q (transposed) ----
            for cb in range(NCB):
                cs = slice(cb * CB, (cb + 1) * CB)
                xk_t = xpool.tile([128, NKC, CB], bf16, tag="xk")
                for half in range(2):
                    hs4 = slice(half * NKC // 2, (half + 1) * NKC // 2)
                    nc.sync.dma_start(xk_t[:, hs4, :], xkv[:, hs4, cs])
                xq_t = xpool.tile([128, NKC, CB], bf16, tag="xq")
                for half in range(2):
                    hs4 = slice(half * NKC // 2, (half + 1) * NKC // 2)
                    nc.sync.dma_start(xq_t[:, hs4, :], xqv[:, hs4, cs])
                if cb == 0:
                    _load_w(wv_sb, wv)
                    nc.sync.dma_start(bv_sb[:], bv.ap())
                    _load_w(wq_sb, wq)
                    nc.sync.dma_start(bq_sb[:], bq.ap())
                    nc.sync.dma_start(ones_row[:], ones.ap())
                for hl in range(HG):
                    hs = slice(hl * 128, (hl + 1) * 128)
                    ps_k = pmm.tile([128, CB], f32, tag="mm")
                    for kc in range(NKC):
                        nc.tensor.matmul(ps_k[:], wk_sb[:, kc, hs], xk_t[:, kc, :],
                                         start=(kc == 0), stop=(kc == NKC - 1))
                    nc.scalar.activation(kh_sb[:, hl, cs], ps_k[:], LRELU,
                                         bias=bk_sb[:, hl:hl + 1], alpha=SLOPE)
                    ps_v = pmm.tile([128, CB], f32, tag="mm")
                    for kc in range(NKC):
                        nc.tensor.matmul(ps_v[:], wv_sb[:, kc, hs], xq_t[:, kc, :],
                                         start=(kc == 0), stop=(kc == NKC - 1))
                    nc.scalar.activation(vh_sb[:, hl, cs], ps_v[:], LRELU,
                                         bias=bv_sb[:, hl:hl + 1], alpha=SLOPE)
                for js in range(CB // 128):
                    ps_q = pmm.tile([128, 512], f32, tag="mm")
                    for kc in range(NKC):
                        nc.tensor.matmul(ps_q[:], xq_t[:, kc, js * 128:(js + 1) * 128],
                                         wq_sb[:, kc, :],
                                         start=(kc == 0), stop=False)
                    nc.tensor.matmul(ps_q[:], ones_row[:, 0:128], bq_sb[:],
                                     start=False, stop=True)
                    nc.scalar.activation(qt_sb[:, cb * (CB // 128) + js, :], ps_q[:],
                                         LRELU, alpha=SLOPE)

            # ---- attention phase, per local head and 1024-wide i-half ----
            # S^T is built 1024 wide (two 512 matmuls sharing one lhsT load)
            # so each exp eviction covers 1024 columns; the second attention
            # matmul reuses each Qt chunk for both 512-wide accumulators.
            for hl in range(HG):
                for ih in range(2):
                    pts = []
                    for jm in range(NJC):
                        ps_s = pmm.tile([128, 1024], f32, tag="mm")
                        for sub in range(2):
                            nc.tensor.matmul(ps_s[:, sub * 512:(sub + 1) * 512],
                                             vh_sb[:, hl, jm * 128:(jm + 1) * 128],
                                             kh_sb[:, hl, ih * 1024 + sub * 512:ih * 1024 + (sub + 1) * 512],
                                             start=True, stop=True)
                        pt = ptpool.tile([128, 1024], bf16, tag="pt")
                        nc.scalar.activation(pt[:], ps_s[:], EXP, scale=sc)
                        pts.append(pt)
                    ps_oa = pout.tile([128, 512], f32, tag="po")
                    ps_ob = pout.tile([128, 512], f32, tag="po")
                    ps_la = psm.tile([1, 512], f32, tag="sm")
                    ps_lb = psm.tile([1, 512], f32, tag="sm")
                    for jc in range(NJC):
                        nc.tensor.matmul(ps_oa[:], qt_sb[:, jc, hl * 128:(hl + 1) * 128],
                                         pts[jc][:, 0:512], start=(jc == 0), stop=(jc == NJC - 1))
                        nc.tensor.matmul(ps_ob[:], qt_sb[:, jc, hl * 128:(hl + 1) * 128],
                                         pts[jc][:, 512:1024], start=(jc == 0), stop=(jc == NJC - 1))
                        nc.tensor.matmul(ps_la[:], ones_col[:], pts[jc][:, 0:512],
                                         start=(jc == 0), stop=(jc == NJC - 1))
                        nc.tensor.matmul(ps_lb[:], ones_col[:], pts[jc][:, 512:1024],
                                         start=(jc == 0), stop=(jc == NJC - 1))
                    for sub, (ps_o, ps_l) in enumerate(((ps_oa, ps_la), (ps_ob, ps_lb))):
                        rb1 = opool.tile([1, 512], f32, tag="rb1")
                        nc.vector.reciprocal_approx_fast(rb1[:], ps_l[:])
                        rbc = opool.tile([128, 512], f32, tag="rbc")
                        nc.gpsimd.partition_broadcast(rbc[:], rb1[:])
                        ot = opool.tile([128, 512], f32, tag="ot")
                        nc.vector.tensor_mul(ot[:], ps_o[:], rbc[:])
                        nc.sync.dma_start(
                            out.ap()[hl * 128:(hl + 1) * 128,
                                     ih * 1024 + sub * 512:ih * 1024 + (sub + 1) * 512],
                            ot[:])

    nc.compile()
    return nc


def _get_program():
    global _PROGRAM
    if _PROGRAM is None:
        _PROGRAM = _build_program()
    return _PROGRAM


def kernel(Xq, Xk, Wq, Wk, Wv,
           gamma1, beta1, mean1, var1,
           gamma2, beta2, mean2, var2,
           gamma3, beta3, mean3, var3):
    global LAST_RESULTS
    from concourse.bass_utils import run_bass_kernel_spmd

    Xq = np.asarray(Xq, np.float32)
    Xk = np.asarray(Xk, np.float32)

    def fold(Wm, gamma, beta, mean, var):
        scale = np.asarray(gamma, np.float32) / np.sqrt(np.asarray(var, np.float32) + EPS)
        bias = np.asarray(beta, np.float32) - np.asarray(mean, np.float32) * scale
        Ws = np.asarray(Wm, np.float32) * scale[:, None]
        return Ws, bias

    Wq_s, b1 = fold(Wq, gamma1, beta1, mean1, var1)
    Wk_s, b2 = fold(Wk, gamma2, beta2, mean2, var2)
    Wv_s, b3 = fold(Wv, gamma3, beta3, mean3, var3)

    import ml_dtypes
    bf = ml_dtypes.bfloat16
    Xq_b = Xq.astype(bf)
    Xk_b = Xk.astype(bf)
    in_maps = []
    for core in range(8):
        b, hg = divmod(core, 2)
        sl = slice(hg * 512, (hg + 1) * 512)
        in_maps.append({
            "xq": np.ascontiguousarray(Xq_b[b]),
            "xk": np.ascontiguousarray(Xk_b[b]),
            "wq": np.ascontiguousarray(Wq_s[sl, :].T.astype(bf)),
            "wk": np.ascontiguousarray(Wk_s[sl, :].T.astype(bf)),
            "wv": np.ascontiguousarray(Wv_s[sl, :].T.astype(bf)),
            "bq": np.ascontiguousarray(b1[sl].reshape(1, 512).astype(bf)),
            "ones": np.ones((1, 128), bf),
            "bk": np.ascontiguousarray(b2[sl].reshape(HG, 128).T),
            "bv": np.ascontiguousarray(b3[sl].reshape(HG, 128).T),
        })

    nc = _get_program()
    trace = os.environ.get("KERNEL_TRACE", "0") == "1"
    res = run_bass_kernel_spmd(nc, in_maps, core_ids=list(range(8)), trace=trace)
    LAST_RESULTS = res

    full = np.empty((B, C, P), np.float32)
    for core in range(8):
        b, hg = divmod(core, 2)
        full[b, hg * 512:(hg + 1) * 512, :] = res.results[core]["out"]
    return full
